# revision 70
# baseline (speedup 1.0000x reference)
"""TRN2 Bass kernel for nn_BlendEmoBackbone: gated audio mixer + low-rank
multiplicative fusion, data-parallel over batch on 8 NeuronCores.

v3 additions (on top of v2 below):
- Mask-run restriction: host sorts each core's batch columns by the 4-bit
  token-mask pattern (order chosen by an exact subset-DP minimizing the
  weighted chain spans) and deals the globally sorted batch round-robin to
  cores, so every LMF chain's valid columns form one compact run shared by
  all 8 SPMD cores. z == 1 outside the run, so the 8 K-tile matmuls per
  (rank, h-tile, chain) stream only ~64% of the columns; the complement
  columns get their constant from the K=2 tail (kt0's start=True marks the
  whole psum bank pending-zero, so the complement tail writes, not adds).
  The program is compiled per run-tuple at first call (cached).
- LN stats quad-packed: the M=1 ones-matmul column sums issue 4-wide into
  distinct PE column groups (tile_position=(0,32g)) and land in rows
  0/32/64/96 of one psum bank (bf16 only — fp32r rejects col groups).
- Audio-token Σx/Σx² rows computed once and reused by all four gates.
- The four K=2 LMF tails issue into distinct PE row groups
  (tile_position=(32*mi,0); uv rows and factor tail rows replicated at
  partitions 32*mi by the host).
- r0 partial-product chains emitted immediately after the blend that
  unblocks them, filling PE gaps under the later gates' serial chains.

Strategy (v2, bf16):
- Pure data parallel: each core handles B/8 = 512 batch rows; gate MLP
  weights and LMF factor tensors replicated (bf16 halves HBM traffic).
- All activations in transposed [feature, batch] layout; every matmul
  contracts over the partition dim. bf16 operands stream 1 cycle/row on
  the PE (fp32/f32r streams at ~2 cycles/row on real TRN2).
- LayerNorm stats via PE ones-matmul column sums; -mu folded into gate
  matmuls as an extra K=1 row.
- LMF where(mask, z, 1) + x_aug ones-column folded into a K=2 tail tile
  in the same psum chain; rank_w folded into the audio factor slices.
- Factors stored partition-major [R,HT,M,P,9,P] so each (r,ht) loads
  with ONE contiguous-per-partition DMA (2.3KB lines).
- WGO gate weights resident in SBUF (single DMA, reused by 3 gates);
  the audio-source half of the other-gate matmul computed once (S).
- Row->tile broadcasts on gpsimd (partition_broadcast); abs/gelu/
  sigmoid/psum-copies on the scalar engine; products/blends on DVE in
  bf16 where precision allows.
"""

import numpy as np
import ml_dtypes
from contextlib import ExitStack

import concourse.bass as bass
from concourse import bacc
import concourse.tile as tile
from concourse import mybir
from concourse.bass_utils import run_bass_kernel_spmd

B, M, H, R = 4096, 4, 1024, 10
NCORES = 8
BS = B // NCORES          # 512 batch rows per core
MID = 512
P = 128
HT = H // P               # 8 h-tiles
MT = MID // P             # 4 mid-tiles
D3 = 3 * H
OTHERS = (0, 2, 3)
AUDIO = 1
EPS = 1e-5

f32 = mybir.dt.float32
f32r = mybir.dt.float32r
bf16 = mybir.dt.bfloat16
u8 = mybir.dt.uint8
AF = mybir.ActivationFunctionType
OP = mybir.AluOpType
bfnp = ml_dtypes.bfloat16

TRACE = False
LAST_RESULTS = None

_cached_nc = {}


def _build(runs, gate_runs, blend_runs):
    """runs[mi] = (lo, hi): column range covering every batch column where
    LMF chain mi is valid (columns are pre-sorted by mask pattern on the
    host). Outside the run z == 1, so the main k-tile matmuls stream only
    [lo:hi); complement segments get their constant via the K=2 tail.
    gate_runs[j] (j=0..2) covers pair_valid_j; gate_runs[3] covers aum —
    gate math outside its run is discarded by the blend, so all gate
    matmuls/DVE work stream only the run columns."""
    nc = bacc.Bacc("TRN2", target_bir_lowering=False, debug=False)
    # hull of the other-gate runs: a2o / S are consumed inside those runs
    ghull = (min(g[0] for g in gate_runs[:3]), max(g[1] for g in gate_runs[:3]))
    arun = gate_runs[3]

    # ---- DRAM parameters (per core) ----
    tokT = nc.declare_dram_parameter("tokT", [M, H, BS], bf16, isOutput=False)
    # u8 rows: 0-2 pv_j, 3 am(aum)
    u8rows = nc.declare_dram_parameter("u8rows", [4, BS], u8, isOutput=False)
    # bf16 rows: 0-2 mo_j, 3 ma, 4-6 cm_j
    f16rows = nc.declare_dram_parameter("f16rows", [7, BS], bf16, isOutput=False)
    # [mask; 1-mask] pairs per chain mi, placed at partitions 32*mi (+0,1)
    # so the four K=2 LMF tail matmuls can pack into distinct PE row groups.
    UVQ = nc.declare_dram_parameter("UVQ", [2 * M, BS], bf16, isOutput=False)
    WGO = nc.declare_dram_parameter("WGO", [3 * HT, P, MID], bf16, isOutput=False)
    WGA = nc.declare_dram_parameter("WGA", [3 * HT, P, MID], bf16, isOutput=False)
    C1 = nc.declare_dram_parameter("C1", [P, MT, 2], f32, isOutput=False)
    W2 = nc.declare_dram_parameter("W2", [P, MT, 2], bf16, isOutput=False)
    CB = nc.declare_dram_parameter("CB", [P, 8], f32, isOutput=False)
    SC = nc.declare_dram_parameter("SC", [1, 8], f32, isOutput=False)
    # [ht_out, kt, P, P] tiled weight blocks (lhsT layout)
    A2OT = nc.declare_dram_parameter("A2OT", [HT, HT, P, P], bf16, isOutput=False)
    O2AT = nc.declare_dram_parameter("O2AT", [HT, HT, P, P], bf16, isOutput=False)
    OUTWT = nc.declare_dram_parameter("OUTWT", [HT, HT, P, P], bf16, isOutput=False)
    # cols: ln_o_w 0:8, ln_o_b 8:16, ln_a_w 16:24, ln_a_b 24:32,
    #       ln1w 32:40, ln1b 40:48, ln2w 48:56, ln2b 56:64, outb 64:72, lmfb 72:80
    LNV = nc.declare_dram_parameter("LNV", [P, 80], f32, isOutput=False)
    # partition-major factor blocks; [.., p, 0:8, :] = main k-tiles,
    # [.., 0:2, 8, :] = [bias_row; ones_or_rankw_row]
    FT = nc.declare_dram_parameter("FT", [R, HT, M, P, 9, P], bf16, isOutput=False)
    OUT = nc.declare_dram_parameter("outT", [H, BS], bf16, isOutput=True)

    with tile.TileContext(nc) as tc, ExitStack() as ctx:
        kp = ctx.enter_context(tc.tile_pool(name="konst", bufs=1))
        tokp = ctx.enter_context(tc.tile_pool(name="tokp", bufs=1))
        big = ctx.enter_context(tc.tile_pool(name="big", bufs=1))
        wk = ctx.enter_context(tc.tile_pool(name="wk", bufs=2))
        bcp = ctx.enter_context(tc.tile_pool(name="bcp", bufs=1))
        sqp = ctx.enter_context(tc.tile_pool(name="sqp", bufs=9))
        sqf = ctx.enter_context(tc.tile_pool(name="sqf", bufs=3))
        wgp = ctx.enter_context(tc.tile_pool(name="wgp", bufs=2))
        ftp = ctx.enter_context(tc.tile_pool(name="ftp", bufs=2))
        rowp = ctx.enter_context(tc.tile_pool(name="rowp", bufs=1))
        # 3 bufs: the LMF chains' psum WAR on the DVE product reads was the
        # top PE stall in the audio-gate stretch (8/8 PSUM banks now used)
        ppz = ctx.enter_context(tc.tile_pool(name="ppz", bufs=3, space="PSUM"))
        ppg = ctx.enter_context(tc.tile_pool(name="ppg", bufs=4, space="PSUM"))
        pps = ctx.enter_context(tc.tile_pool(name="pps", bufs=1, space="PSUM"))

        # ---- constants / small loads ----
        ones_k = kp.tile([P, 1], bf16)
        nc.vector.memset(ones_k, 1.0)
        ones_kf32 = kp.tile([P, 1], f32)
        nc.vector.memset(ones_kf32, 1.0)
        ones_kf = ones_kf32.bitcast(f32r)

        def bc_row_dma(dst, src_ap):
            nc.sync.dma_start(
                out=dst,
                in_=bass.AP(
                    tensor=src_ap.tensor, offset=src_ap.offset, ap=[[0, P], [1, BS]]
                ),
            )

        # ---- tokens (transposed, bf16); audio first so a2o starts early;
        # tokens 2,3 deferred past the first weight loads (needed later) ----
        tok = tokp.tile([P, M, HT, BS], bf16)
        for m in (AUDIO, 0):
            nc.sync.dma_start(
                out=tok[:, m], in_=tokT.ap()[m].rearrange("(ht p) b -> p ht b", p=P)
            )

        u8t = []
        for i in range(4):
            t = kp.tile([P, BS], u8, tag=f"u8_{i}")
            bc_row_dma(t, u8rows.ap()[i : i + 1, :])
            u8t.append(t)
        pv_t, am_t = u8t[0:3], u8t[3]
        f16t = []
        for i in range(7):
            t = kp.tile([P, BS], bf16, tag=f"f16_{i}")
            bc_row_dma(t, f16rows.ap()[i : i + 1, :])
            f16t.append(t)
        mo_t, ma_t, cm_t = f16t[0:3], f16t[3], f16t[4:7]
        # uvq: chain mi's [mask; 1-mask] rows at partitions 32*mi, 32*mi+1
        uvq = kp.tile([P, BS], bf16)
        for mi in range(M):
            for j in range(2):
                nc.sync.dma_start(
                    out=uvq[32 * mi + j : 32 * mi + j + 1, :],
                    in_=UVQ.ap()[2 * mi + j : 2 * mi + j + 1, :],
                )

        def uvr(mi):
            return uvq[32 * mi : 32 * mi + 2, :]
        cbt = kp.tile([P, 8], f32)
        nc.sync.dma_start(out=cbt, in_=CB.ap())
        sct = kp.tile([1, 8], f32)
        nc.sync.dma_start(out=sct, in_=SC.ap())
        lnv = kp.tile([P, 80], f32)
        nc.sync.dma_start(out=lnv, in_=LNV.ap())
        w2t = kp.tile([P, MT, 2], bf16)
        nc.sync.dma_start(out=w2t, in_=W2.ap())
        c1t = kp.tile([P, MT, 2], f32)
        nc.sync.dma_start(out=c1t, in_=C1.ap())

        def tk(m, kt):
            return tok[:, m, kt, :]

        def tkw(m):  # whole-token [P, HT, BS] view
            return tok[:, m]

        def flat(t3):
            return t3.rearrange("p a b -> p (a b)")

        def b3(t2):  # [P,BS] -> broadcast [P,HT,BS]
            return t2.unsqueeze(1).broadcast_to([P, HT, BS])

        # ---- quad-packed LN stats ----
        # statQ rows 0/32/64/96 are independent psum accumulators; chunk
        # matmuls are issued into distinct PE column groups (tile_position)
        # so up to 4 of them execute concurrently in the array.
        QROW = (0, 32, 64, 96)

        def quad_alloc(name):
            return pps.tile([97, BS], f32, tag="statQ", name=name)

        def quad_mm(statQ, g, rhs, start, stop, lhs=None, cols=(0, BS)):
            r = QROW[g]
            lo, hi = cols
            nc.tensor.matmul(
                statQ[r : r + 1, lo:hi], ones_k if lhs is None else lhs, rhs,
                start=start, stop=stop, tile_position=(0, r),
            )

        def quad_finish(statQ, extraA=None, extraB=None, cols=(0, BS)):
            # DVE may read at most one PSUM operand per instruction
            lo, hi = cols
            rowA = rowp.tile([1, BS], f32, tag="qrowA")
            rowB = rowp.tile([1, BS], f32, tag="qrowB")
            if extraA is not None:
                nc.vector.tensor_add(rowA[:, lo:hi], statQ[0:1, lo:hi], extraA[:, lo:hi])
            else:
                nc.scalar.activation(rowA[:, lo:hi], statQ[0:1, lo:hi], AF.Copy)
            nc.vector.tensor_add(rowA[:, lo:hi], rowA[:, lo:hi], statQ[32:33, lo:hi])
            if extraB is not None:
                nc.vector.tensor_add(rowB[:, lo:hi], statQ[64:65, lo:hi], extraB[:, lo:hi])
            else:
                nc.scalar.activation(rowB[:, lo:hi], statQ[64:65, lo:hi], AF.Copy)
            nc.vector.tensor_add(rowB[:, lo:hi], rowB[:, lo:hi], statQ[96:97, lo:hi])
            return rowA, rowB

        # ---- helpers ----
        def ln_rows(statA, statB, n, tag, hi_mu=False, par=0, cols=(0, BS)):
            lo, hi = cols
            mdt = f32 if hi_mu else bf16
            mtag = "negmuf" if hi_mu else f"negmu{par}"
            negmu = rowp.tile([1, BS], mdt, tag=mtag, name=f"negmu_{tag}")
            nc.scalar.activation(negmu[:, lo:hi], statA[0:1, lo:hi], AF.Copy, bias=0.0, scale=-1.0 / n)
            ex2 = rowp.tile([1, BS], f32, tag="ex2", name=f"ex2_{tag}")
            nc.scalar.activation(ex2[:, lo:hi], statB[0:1, lo:hi], AF.Copy, bias=0.0, scale=1.0 / n)
            msq = rowp.tile([1, BS], f32, tag="msq", name=f"msq_{tag}")
            nc.scalar.activation(msq[:, lo:hi], negmu[:, lo:hi], AF.Square)
            nc.vector.tensor_sub(ex2[:, lo:hi], ex2[:, lo:hi], msq[:, lo:hi])
            nc.vector.tensor_scalar_max(ex2[:, lo:hi], ex2[:, lo:hi], 0.0)
            # 1/sqrt(var+eps) = exp(-0.5*ln(var+eps)); ln+exp share one ACT
            # table so this replaces the 3.3us DVE reciprocal with two fast
            # scalar-engine ops off the DVE queue.
            nc.scalar.activation(msq[:, lo:hi], ex2[:, lo:hi], AF.Ln, bias=sct[0:1, 2:3], scale=1.0)
            rinvb = rowp.tile([1, BS], bf16, tag=f"rinvb{par}", name=f"rinvb_{tag}")
            nc.scalar.activation(rinvb[:, lo:hi], msq[:, lo:hi], AF.Exp, bias=0.0, scale=-0.5)
            return negmu, rinvb

        def bcast(row, tag, dt=bf16, cols=(0, BS)):
            """Broadcast a [1,*] row to [P,*] via gpsimd."""
            lo, hi = cols
            sb = bcp.tile([P, BS], dt, tag=f"bc_{tag}")
            nc.gpsimd.partition_broadcast(sb[:, lo:hi], row[0:1, lo:hi])
            return sb

        def b3c(t2, lo, hi):  # [P,BS] row-tile slice -> [P,HT,cols] bcast view
            return t2[:, lo:hi].unsqueeze(1).broadcast_to([P, HT, hi - lo])

        def linmap(WT, src3, dst3, cols=(0, BS)):
            """dst3[ho] = sum_kt WT[ho,kt].T @ src3[kt]; WT streamed from DRAM."""
            lo, hi = cols
            for ho in range(HT):
                wt = wgp.tile([P, HT, P], bf16, tag="lin")
                nc.sync.dma_start(out=wt, in_=WT.ap()[ho].rearrange("k p c -> p k c"))
                ps = ppz.tile([P, BS], f32, tag="z")
                for kt in range(HT):
                    nc.tensor.matmul(
                        ps[:, lo:hi], wt[:, kt, :], src3[:, kt, lo:hi],
                        start=(kt == 0), stop=(kt == HT - 1),
                    )
                nc.scalar.activation(dst3[:, ho, lo:hi], ps[:, lo:hi], AF.Copy)

        # ---- cached audio column stats (Σx, Σx²), shared by all 4 gates;
        # also primes the PE while the weight DMAs land ----
        csQ = quad_alloc("csQ")
        for k in range(HT):
            asq = sqp.tile([P, BS], bf16, tag="sq_sq", name=f"asq{k}")
            nc.vector.tensor_mul(asq, tk(AUDIO, k), tk(AUDIO, k))
            quad_mm(csQ, k % 2, tk(AUDIO, k), start=(k < 2), stop=(k >= HT - 2))
            quad_mm(csQ, 2 + k % 2, asq, start=(k < 2), stop=(k >= HT - 2))
        cs_a = kp.tile([1, BS], f32)
        ss_a = kp.tile([1, BS], f32)
        nc.scalar.activation(cs_a, csQ[0:1, :], AF.Copy)
        nc.vector.tensor_add(cs_a, cs_a, csQ[32:33, :])
        nc.scalar.activation(ss_a, csQ[64:65, :], AF.Copy)
        nc.vector.tensor_add(ss_a, ss_a, csQ[96:97, :])

        # ---- a2o = audio @ a2o_w.T, in T layout (bf16) ----
        a2or = big.tile([P, HT, BS], bf16, tag="axr")
        linmap(A2OT, tkw(AUDIO), a2or, cols=ghull)

        # S_mt = sum_k Wgo_s[k].T @ audio  (shared source half of gate1)
        hlo, hhi = ghull
        S = big.tile([P, MT, BS], bf16, tag="Sg")
        for mt in range(MT):
            wS = wgp.tile([P, HT, P], bf16, tag="lin", name=f"wS{mt}")
            nc.sync.dma_start(
                out=wS,
                in_=WGO.ap()[HT : 2 * HT, :, mt * P : (mt + 1) * P].rearrange(
                    "k p c -> p k c"
                ),
            )
            ps = ppz.tile([P, BS], f32, tag="z")
            for k in range(HT):
                nc.tensor.matmul(
                    ps[:, hlo:hhi], wS[:, k, :], tk(AUDIO, k)[:, hlo:hhi],
                    start=(k == 0), stop=(k == HT - 1),
                )
            nc.scalar.activation(S[:, mt, hlo:hhi], ps[:, hlo:hhi], AF.Copy)

        # deferred token loads (queued behind the first gate's weights)
        for m in (2, 3):
            nc.sync.dma_start(
                out=tok[:, m], in_=tokT.ap()[m].rearrange("(ht p) b -> p ht b", p=P)
            )

        omt = big.tile([P, HT, BS], bf16, tag="om")  # others_mean accumulator
        mix_src = {"x": a2or}

        def gate_phase1(j, mj):
            """Stats + gate1 matmuls + LN rows for gate j. Emission order
            keeps the PE fed: gate1 halves (no stats dependency) are
            interleaved with the DVE-paced stat chains."""
            is_audio = j == 3
            glo, ghi = gate_runs[j]
            t_m = AUDIO if is_audio else mj
            t3 = tkw(t_m)
            s3 = omt if is_audio else tkw(AUDIO)
            x3 = s3 if is_audio else t3  # the non-cached (non-audio) operand

            abs3 = big.tile([P, HT, BS], bf16, tag="abs", name=f"abs3_{j}")
            statQ = quad_alloc(f"statQ_{j}")
            gps = [
                ppg.tile([P, BS], f32, tag="g", name=f"gps{j}_{mt}")
                for mt in range(MT)
            ]
            if is_audio:
                parts = [(WGA, 0, t3), (WGA, 1, s3), (WGA, 2, abs3)]
            else:
                parts = [(WGO, 0, t3), (WGO, 2, abs3)]

            # DVE production first: d, |d| (ACT), d^2 tiles
            dsq = []
            for k in range(HT):
                dk = wk.tile([P, BS], bf16, tag="dk")
                nc.vector.tensor_sub(dk[:, glo:ghi], t3[:, k, glo:ghi], s3[:, k, glo:ghi])
                nc.scalar.activation(abs3[:, k, glo:ghi], dk[:, glo:ghi], AF.Abs)
                sq = sqp.tile([P, BS], bf16, tag="sq_sq")
                nc.vector.tensor_mul(sq[:, glo:ghi], dk[:, glo:ghi], dk[:, glo:ghi])
                dsq.append(sq)

            def mt_chain(mt):
                for pi, (WG, part, rhs3) in enumerate(parts):
                    w = wgp.tile([P, HT, P], bf16, tag="lin", name=f"wg{j}_{mt}_{part}")
                    nc.sync.dma_start(
                        out=w,
                        in_=WG.ap()[
                            part * HT : (part + 1) * HT, :, mt * P : (mt + 1) * P
                        ].rearrange("k p c -> p k c"),
                    )
                    for k in range(HT):
                        nc.tensor.matmul(
                            gps[mt][:, glo:ghi], w[:, k, :], rhs3[:, k, glo:ghi],
                            start=(pi == 0 and k == 0),
                            stop=(pi == len(parts) - 1 and k == HT - 1),
                        )

            # quad stats: g0=x3 (A), g1=abs3 (A), g2=d^2 (B), g3=x3^2 (B);
            # the audio-token Σx/Σx² parts come from the cached rows.
            def stat_slots(ks):
                for k in ks:
                    sq = sqp.tile([P, BS], bf16, tag="sq_sq", name=f"xsq{j}_{k}")
                    nc.vector.tensor_mul(sq[:, glo:ghi], x3[:, k, glo:ghi], x3[:, k, glo:ghi])
                    quad_mm(statQ, 0, x3[:, k, glo:ghi], start=(k == 0), stop=(k == HT - 1), cols=(glo, ghi))
                    quad_mm(statQ, 1, abs3[:, k, glo:ghi], start=(k == 0), stop=(k == HT - 1), cols=(glo, ghi))
                    quad_mm(statQ, 2, dsq[k][:, glo:ghi], start=(k == 0), stop=(k == HT - 1), cols=(glo, ghi))
                    quad_mm(statQ, 3, sq[:, glo:ghi], start=(k == 0), stop=(k == HT - 1), cols=(glo, ghi))

            mt_chain(0)
            mt_chain(1)
            stat_slots(range(0, HT // 2))
            mt_chain(2)
            stat_slots(range(HT // 2, HT))
            mt_chain(3)
            rowA, rowB = quad_finish(statQ, extraA=cs_a, extraB=ss_a, cols=(glo, ghi))
            negmu, rinvb = ln_rows(rowA, rowB, D3, f"g{j}", par=j % 2, cols=(glo, ghi))
            return abs3, negmu, rinvb, gps

        def gate_phase2a(j, mj, abs3, negmu, rinvb, gps):
            """Gate layer 2 + mix pre-activation for gate j."""
            is_audio = j == 3
            glo, ghi = gate_runs[j]
            t3 = tkw(AUDIO if is_audio else mj)
            rb = bcast(rinvb, "rb", cols=(glo, ghi))
            nmb = bcast(negmu, "nm", cols=(glo, ghi))
            cb_off = 4 if is_audio else 0
            col = 1 if is_audio else 0
            gp = pps.tile([1, BS], f32, tag="statQ", name=f"gp{j}")
            for mt in range(MT):
                hm = wk.tile([P, BS], f32, tag="hm")
                # hm = gps + (-mu)*c1 [+ S]; then * rinv
                nc.vector.scalar_tensor_tensor(
                    hm[:, glo:ghi], nmb[:, glo:ghi], c1t[:, mt, col : col + 1],
                    gps[mt][:, glo:ghi], op0=OP.mult, op1=OP.add,
                )
                if not is_audio:
                    nc.vector.tensor_add(hm[:, glo:ghi], hm[:, glo:ghi], S[:, mt, glo:ghi])
                nc.vector.tensor_mul(hm[:, glo:ghi], hm[:, glo:ghi], rb[:, glo:ghi])
                hg1 = wk.tile([P, BS], bf16, tag="hg", name=f"hg{mt}")
                nc.scalar.activation(
                    hg1[:, glo:ghi], hm[:, glo:ghi], AF.Gelu,
                    bias=cbt[:, cb_off + mt : cb_off + mt + 1], scale=1.0,
                )
                nc.tensor.matmul(
                    gp[0:1, glo:ghi], w2t[:, mt, col : col + 1], hg1[:, glo:ghi],
                    start=(mt == 0), stop=(mt == MT - 1),
                )
            g_row = rowp.tile([1, BS], bf16, tag="g_row")
            nc.scalar.activation(
                g_row[:, glo:ghi], gp[0:1, glo:ghi], AF.Sigmoid,
                bias=sct[0:1, col : col + 1], scale=1.0,
            )
            gb = bcast(g_row, "gb", cols=(glo, ghi))
            # pre = t + g * (a2o | o2a)
            src = mix_src["x"]
            pre = big.tile([P, HT, BS], bf16, tag=f"pre{j % 2}", name=f"pre{j}")
            nc.vector.tensor_mul(pre[:, :, glo:ghi], src[:, :, glo:ghi], b3c(gb, glo, ghi))
            nc.vector.tensor_add(pre[:, :, glo:ghi], pre[:, :, glo:ghi], t3[:, :, glo:ghi])
            return pre

        def gate_phase2b(j, mj, pre):
            """Mix LN + blend for gate j."""
            is_audio = j == 3
            glo, ghi = gate_runs[j]
            alo, ahi = arun
            t_m = AUDIO if is_audio else mj
            t3 = tkw(t_m)
            stat2Q = quad_alloc(f"stat2Q_{j}")
            for k in range(HT):
                sq = sqp.tile([P, BS], bf16, tag="sq_sq", name=f"psq{j}_{k}")
                nc.vector.tensor_mul(sq[:, glo:ghi], pre[:, k, glo:ghi], pre[:, k, glo:ghi])
                quad_mm(stat2Q, k % 2, pre[:, k, glo:ghi], start=(k < 2), stop=(k >= HT - 2), cols=(glo, ghi))
                quad_mm(stat2Q, 2 + k % 2, sq[:, glo:ghi], start=(k < 2), stop=(k >= HT - 2), cols=(glo, ghi))
            rowA2, rowB2 = quad_finish(stat2Q, cols=(glo, ghi))
            negmu2, rinvb2 = ln_rows(rowA2, rowB2, H, f"u{j}", par=2 + (j % 2), cols=(glo, ghi))
            mb = bcast(negmu2, "mb", cols=(glo, ghi))
            rb2 = bcast(rinvb2, "rb2", cols=(glo, ghi))
            wcol = 16 if is_audio else 0
            bcol = 24 if is_audio else 8
            sm = am_t if is_audio else pv_t[j]
            bmf = ma_t if is_audio else mo_t[j]
            # whole-token LN apply + blend: tok = bmf * (sm ? ln(pre) : t).
            # q3 is only ever SELECTED where sm (pair_valid) is set, and the
            # sorted columns confine sm to blend_runs[j] — so the LN apply
            # and the predicated copy run only that span (pure DVE relief;
            # the gate matmuls stay full width as latency filler).
            qlo, qhi = blend_runs[j]
            q3 = big.tile([P, HT, BS], bf16, tag="q3", name=f"q3_{j}")
            nc.vector.tensor_add(q3[:, :, qlo:qhi], pre[:, :, qlo:qhi], b3c(mb, qlo, qhi))
            nc.vector.tensor_mul(q3[:, :, qlo:qhi], q3[:, :, qlo:qhi], b3c(rb2, qlo, qhi))
            for kt in range(HT):
                nc.vector.tensor_scalar(
                    q3[:, kt, qlo:qhi], q3[:, kt, qlo:qhi],
                    lnv[:, wcol + kt : wcol + kt + 1], lnv[:, bcol + kt : bcol + kt + 1],
                    op0=OP.mult, op1=OP.add,
                )
            nc.vector.copy_predicated(t3[:, :, qlo:qhi], b3c(sm, qlo, qhi), q3[:, :, qlo:qhi])
            nc.vector.tensor_mul(t3, t3, b3(bmf))
            if not is_audio:
                # others_mean is only consumed inside the aum run
                if j == 0:
                    nc.vector.tensor_mul(omt[:, :, alo:ahi], t3[:, :, alo:ahi], b3c(cm_t[j], alo, ahi))
                else:
                    tmp3 = big.tile([P, HT, BS], bf16, tag="q3", name=f"omtmp_{j}")
                    nc.vector.tensor_mul(tmp3[:, :, alo:ahi], t3[:, :, alo:ahi], b3c(cm_t[j], alo, ahi))
                    nc.vector.tensor_add(omt[:, :, alo:ahi], omt[:, :, alo:ahi], tmp3[:, :, alo:ahi])

        # LMF chain-index order: factors stored with M reordered as MORD so
        # the audio (blended last) chain comes last; r=0 partial products
        # for the non-audio tokens are emitted between mixer phases to keep
        # the PE busy during the gates' serial post-chains.
        MORD = (0, 2, 3, 1)
        acc = big.tile([P, HT, BS], f32r, tag="acc")
        soth = big.tile([P, HT, BS], bf16, tag="soth")

        def chain_tails(zp, ft_tail, mi, comps=True):
            """K=2 tail over the run (accumulate) plus complement segments
            (first write there — kt0's start marked the bank pending-zero,
            so these overwrite with mask*bias + (1-mask)*1 == 1). comps=False
            skips the complement when no consumer reads those columns."""
            lo, hi = runs[mi]
            segs = [(lo, hi)]
            if comps and lo > 0:
                segs.append((0, lo))
            if comps and hi < BS:
                segs.append((hi, BS))
            for si, (s, e) in enumerate(segs):
                nc.tensor.matmul(
                    zp[:, s:e], ft_tail, uvr(mi)[:, s:e],
                    start=False, stop=(si == len(segs) - 1),
                    tile_position=(32 * mi, 0),
                )

        def lmf_part_chain(r, mi, dst):
            """Partial-product chains for rank r emitted out of band: dst
            accumulates z products for chain mi; mi==3 folds into acc."""
            lo, hi = runs[mi]
            for ht in range(HT):
                ft = ftp.tile([P, 9, P], bf16, tag="ft0", bufs=6)
                nc.sync.dma_start(out=ft, in_=FT.ap()[r, ht, mi])
                zp = ppz.tile([P, BS], f32, tag="z")
                for kt in range(HT):
                    nc.tensor.matmul(
                        zp[:, lo:hi], ft[:, kt, :], tk(MORD[mi], kt)[:, lo:hi],
                        start=(kt == 0), stop=False,
                    )
                chain_tails(zp, ft[32 * mi : 32 * mi + 2, 8, :], mi,
                            comps=(mi in (0, 3)))
                if mi == 0:
                    nc.scalar.activation(dst[:, ht, :], zp, AF.Copy)
                elif mi < 3:
                    # in-place product: z == 1 outside this chain's run, so
                    # skipping those columns preserves the running product
                    # (NOT valid for the audio chain: rank_w is folded there)
                    nc.vector.tensor_mul(dst[:, ht, lo:hi], dst[:, ht, lo:hi],
                                         zp[:, lo:hi])
                elif r == 0:
                    nc.vector.tensor_mul(acc[:, ht, :], dst[:, ht, :], zp)
                else:
                    s0 = wk.tile([P, BS], f32, tag="s0")
                    nc.vector.tensor_mul(s0, dst[:, ht, :], zp)
                    nc.vector.tensor_add(acc[:, ht, :], acc[:, ht, :], s0)

        # software-pipelined emission: gate j+1's stats+gate1 overlap gate
        # j's post-matmul chain on the PE; LMF r=0 chains fill blend windows.
        p1, p2 = {}, {}
        p1[0] = gate_phase1(0, OTHERS[0])
        p1[1] = gate_phase1(1, OTHERS[1])
        p2[0] = gate_phase2a(0, OTHERS[0], *p1[0])
        p1[2] = gate_phase1(2, OTHERS[2])
        p2[1] = gate_phase2a(1, OTHERS[1], *p1[1])
        gate_phase2b(0, OTHERS[0], p2[0])
        # r0 chains emitted as soon as their token is blended: they are the
        # PE filler for the later gates' serial LN/broadcast chains.
        lmf_part_chain(0, 0, soth)
        p2[2] = gate_phase2a(2, OTHERS[2], *p1[2])
        gate_phase2b(1, OTHERS[1], p2[1])
        lmf_part_chain(0, 1, soth)
        gate_phase2b(2, OTHERS[2], p2[2])

        # ---- o2a = others_mean @ o2a_w.T (consumed inside the aum run) ----
        o2ar = big.tile([P, HT, BS], bf16, tag="axr", name="o2ar")
        linmap(O2AT, omt, o2ar, cols=arun)
        mix_src["x"] = o2ar

        p1[3] = gate_phase1(3, AUDIO)
        lmf_part_chain(0, 2, soth)
        p2[3] = gate_phase2a(3, AUDIO, *p1[3])
        # r=1 partial products fill the audio post-chain/blend window
        soth1 = big.tile([P, HT, BS], bf16, tag="axr", name="soth1")
        lmf_part_chain(1, 0, soth1)
        gate_phase2b(3, AUDIO, p2[3])
        lmf_part_chain(1, 1, soth1)
        lmf_part_chain(1, 2, soth1)
        lmf_part_chain(0, 3, soth)
        lmf_part_chain(1, 3, soth1)

        # ---- LMF ranks 2..R-1; LN1 stats interleaved into the last rank ----
        # (fp32r matmuls reject col-group tiling, so A/B go to two banks)
        stat3A = pps.tile([1, BS], f32, tag="statQ", name="stat3A")
        stat3B = ppz.tile([1, BS], f32, tag="z", name="stat3B")
        acb = big.tile([P, HT, BS], bf16, tag="soth", name="acb")

        def stat3_for(ht):
            nc.vector.tensor_scalar_add(
                acc[:, ht, :], acc[:, ht, :], lnv[:, 72 + ht : 72 + ht + 1]
            )
            sq = sqf.tile([P, BS], f32r, tag="sq_f")
            nc.vector.tensor_mul(sq, acc[:, ht, :], acc[:, ht, :])
            nc.scalar.activation(acb[:, ht, :], acc[:, ht, :], AF.Copy)
            nc.tensor.matmul(stat3A, ones_kf, acc[:, ht, :],
                             start=(ht == 0), stop=(ht == HT - 1))
            nc.tensor.matmul(stat3B, ones_kf, sq,
                             start=(ht == 0), stop=(ht == HT - 1))

        for r in range(2, R):
            last = r == R - 1
            for ht in range(HT):
                # per-chain factor DMAs: 4x queue parallelism and chain m's
                # matmuls only wait on their own quarter of the load
                fts = []
                for m in range(M):
                    ftm = ftp.tile([P, 9, P], bf16, tag="ft", bufs=8,
                                   name=f"ft{r}_{ht}_{m}")
                    nc.sync.dma_start(out=ftm, in_=FT.ap()[r, ht, m])
                    fts.append(ftm)
                zps = []
                for m in range(M):
                    zp = ppg.tile([P, BS], f32, tag="g", name=f"zp{r}_{ht}_{m}")
                    lo, hi = runs[m]
                    for kt in range(HT):
                        nc.tensor.matmul(
                            zp[:, lo:hi], fts[m][:, kt, :],
                            tk(MORD[m], kt)[:, lo:hi],
                            start=(kt == 0), stop=False,
                        )
                    zps.append(zp)
                # the K=2 tails pack into distinct PE row groups
                for m in range(M):
                    chain_tails(zps[m], fts[m][32 * m : 32 * m + 2, 8, :], m,
                                comps=(m in (0, 3)))
                s0 = wk.tile([P, BS], f32, tag="s0")
                nc.scalar.activation(s0, zps[0], AF.Copy)
                for m in (1, 2):
                    # z == 1 outside runs[m]: skip (also frees psum bank m
                    # sooner); NOT valid for m=3 where z == rank_w outside
                    mlo, mhi = runs[m]
                    nc.vector.tensor_mul(s0[:, mlo:mhi], s0[:, mlo:mhi],
                                         zps[m][:, mlo:mhi])
                nc.vector.tensor_mul(s0, s0, zps[3])
                nc.vector.tensor_add(acc[:, ht, :], acc[:, ht, :], s0)
                if last and ht >= 1:
                    stat3_for(ht - 1)
        stat3_for(HT - 1)
        negmu3, rinvb3 = ln_rows(stat3A, stat3B, H, "l1")
        mb3 = bcast(negmu3, "mb")
        rb3 = bcast(rinvb3, "rb2")
        h1 = big.tile([P, HT, BS], bf16, tag="pre0", name="h1")
        nc.vector.tensor_add(h1, acb, b3(mb3))
        nc.vector.tensor_mul(h1, h1, b3(rb3))
        for kt in range(HT):
            nc.vector.tensor_scalar(
                h1[:, kt, :], h1[:, kt, :],
                lnv[:, 32 + kt : 32 + kt + 1], lnv[:, 40 + kt : 40 + kt + 1],
                op0=OP.mult, op1=OP.add,
            )

        # h2 = gelu(h1 @ out_w.T + out_b); LN2 stats interleaved per ho
        h2 = big.tile([P, HT, BS], bf16, tag="abs", name="h2")
        stat4Q = quad_alloc("stat4Q")
        for ho in range(HT):
            wt = wgp.tile([P, HT, P], bf16, tag="lin", name=f"ow{ho}")
            nc.sync.dma_start(out=wt, in_=OUTWT.ap()[ho].rearrange("k p c -> p k c"))
            ps = ppz.tile([P, BS], f32, tag="z")
            for kt in range(HT):
                nc.tensor.matmul(
                    ps, wt[:, kt, :], h1[:, kt, :],
                    start=(kt == 0), stop=(kt == HT - 1),
                )
            nc.scalar.activation(
                h2[:, ho, :], ps, AF.Gelu, bias=lnv[:, 64 + ho : 64 + ho + 1], scale=1.0
            )
            sq4 = sqp.tile([P, BS], bf16, tag="sq_sq", name=f"hsq{ho}")
            nc.vector.tensor_mul(sq4, h2[:, ho, :], h2[:, ho, :])
            quad_mm(stat4Q, ho % 2, h2[:, ho, :], start=(ho < 2), stop=(ho >= HT - 2))
            quad_mm(stat4Q, 2 + ho % 2, sq4, start=(ho < 2), stop=(ho >= HT - 2))
        rowA4, rowB4 = quad_finish(stat4Q)
        negmu4, rinvb4 = ln_rows(rowA4, rowB4, H, "l2")
        mb4 = bcast(negmu4, "mb")
        rb4 = bcast(rinvb4, "rb2")
        fin3 = big.tile([P, HT, BS], bf16, tag="q3", name="fin3")
        nc.vector.tensor_add(fin3, h2, b3(mb4))
        nc.vector.tensor_mul(fin3, fin3, b3(rb4))
        for kt in range(HT):
            nc.vector.tensor_scalar(
                fin3[:, kt, :], fin3[:, kt, :],
                lnv[:, 48 + kt : 48 + kt + 1], lnv[:, 56 + kt : 56 + kt + 1],
                op0=OP.mult, op1=OP.add,
            )
            nc.sync.dma_start(out=OUT.ap()[kt * P : (kt + 1) * P, :], in_=fin3[:, kt, :])

    nc.compile()
    return nc


MORD_HOST = (0, 2, 3, 1)  # kernel chain order (audio last)


def _optimal_order(wp, chain_defs, cw):
    """Order the 16 mask patterns to minimize the weighted sum of chain
    column spans (exact subset DP, maximizes prefix/suffix zero weight)."""
    NP = 16
    FULL = (1 << NP) - 1
    f = [-1.0] * (1 << NP)
    f[0] = 0.0
    parent = [-1] * (1 << NP)
    for S in range(1 << NP):
        if f[S] < 0:
            continue
        base = f[S]
        for p in range(NP):
            bit = 1 << p
            if S & bit:
                continue
            g = 0.0
            for c, Bc in enumerate(chain_defs):
                if not (Bc >> p) & 1:
                    if (S & Bc) == 0 or (S & Bc) == Bc:
                        g += cw[c] * wp[p]
            nS = S | bit
            v = base + g
            if v > f[nS]:
                f[nS] = v
                parent[nS] = p
    order = []
    S = FULL
    while S:
        p = parent[S]
        order.append(p)
        S &= ~(1 << p)
    order.reverse()
    return order


def _plan_columns(token_mask):
    """Sort batch columns by mask pattern (dealt round-robin to cores) so
    each LMF chain's valid columns sit in one compact run per core.
    Returns (perm[BS, NCORES] global indices, runs, gate_runs)."""
    pat = np.zeros(B, dtype=np.int64)
    for mi, m in enumerate(MORD_HOST):
        pat |= np.asarray(token_mask)[:, m].astype(np.int64) << mi
    wp = (np.bincount(pat, minlength=16).astype(np.float64) / B).tolist()
    chain_defs = [sum(1 << p for p in range(16) if (p >> mi) & 1) for mi in range(4)]
    cw = [720.0] * 4
    for j in range(3):  # gate pv_j = chain j valid & audio(chain 3) valid
        chain_defs.append(
            sum(1 << p for p in range(16) if ((p >> j) & 1) and ((p >> 3) & 1))
        )
        cw.append(70.0)
    chain_defs.append(sum(1 << p for p in range(16) if ((p >> 3) & 1) and (p & 7)))
    cw.append(140.0)
    pord = _optimal_order(wp, chain_defs, cw)
    prio = np.zeros(16, dtype=np.int64)
    for pos, p in enumerate(pord):
        prio[p] = pos
    G = np.argsort(prio[pat], kind="stable")
    perm = G.reshape(BS, NCORES)

    def runspan(valid):
        lo = min(int(np.argmax(valid[:, c])) for c in range(NCORES))
        hi = max(BS - int(np.argmax(valid[::-1, c])) for c in range(NCORES))
        return (lo, hi)

    pv = pat[perm]
    runs = tuple(runspan((pv >> mi) & 1) for mi in range(4))
    gate_runs = tuple(
        runspan(((pv >> j) & 1) & ((pv >> 3) & 1)) for j in range(3)
    ) + (runspan(((pv >> 3) & 1) & (pv & 7 > 0)),)
    return perm, runs, gate_runs


def _host_prep(inputs):
    tokens = np.asarray(inputs["tokens"], np.float32)
    token_mask = np.asarray(inputs["token_mask"])
    mask_f = token_mask.astype(np.float32)
    perm, runs, gate_runs = _plan_columns(token_mask)

    mo = mask_f[:, list(OTHERS)]                      # [B,3]
    ma = mask_f[:, AUDIO]                             # [B]
    pv = mo * ma[:, None]                             # [B,3]
    winv = (1.0 / np.clip(mo.sum(1), 1.0, None)).astype(np.float32)
    aum = ma * (mo.max(1) > 0)                        # [B]

    go_w1 = np.asarray(inputs["go_w1"], np.float32)
    ga_w1 = np.asarray(inputs["ga_w1"], np.float32)

    def gate_prep(w1, b1, lnw, lnb):
        W1w = w1 * lnw[None, :]                       # [MID, 3H]
        c1 = np.ascontiguousarray(W1w.sum(1).reshape(1, MID))
        cb = w1 @ lnb + b1                            # [MID]
        Wblocks = np.ascontiguousarray(W1w.T).reshape(3 * HT, P, MID)
        return Wblocks, c1, cb

    WGOv, c1go, cbgo = gate_prep(
        go_w1, np.asarray(inputs["go_b1"], np.float32),
        np.asarray(inputs["ln_go_w"], np.float32), np.asarray(inputs["ln_go_b"], np.float32),
    )
    WGAv, c1ga, cbga = gate_prep(
        ga_w1, np.asarray(inputs["ga_b1"], np.float32),
        np.asarray(inputs["ln_ga_w"], np.float32), np.asarray(inputs["ln_ga_b"], np.float32),
    )
    CBv = np.ascontiguousarray(
        np.concatenate([cbgo.reshape(MT, P).T, cbga.reshape(MT, P).T], axis=1)
    ).astype(np.float32)                              # [P, 8]
    W2v = np.stack(
        [np.asarray(inputs["go_w2"], np.float32).reshape(MID),
         np.asarray(inputs["ga_w2"], np.float32).reshape(MID)], axis=1
    )                                                 # [MID, 2]
    W2v = np.ascontiguousarray(W2v.reshape(MT, P, 2).transpose(1, 0, 2))
    C1v = np.stack([c1go.reshape(MID), c1ga.reshape(MID)], axis=1)
    C1v = np.ascontiguousarray(C1v.reshape(MT, P, 2).transpose(1, 0, 2)).astype(np.float32)
    SCv = np.zeros((1, 8), np.float32)
    SCv[0, 0] = np.asarray(inputs["go_b2"], np.float32).reshape(-1)[0]
    SCv[0, 1] = np.asarray(inputs["ga_b2"], np.float32).reshape(-1)[0]
    SCv[0, 2] = EPS

    def tile_blocks(w):
        wt = np.ascontiguousarray(np.asarray(w, np.float32).T)    # [H_in, H_out]
        return np.ascontiguousarray(
            wt.reshape(HT, P, HT, P).transpose(2, 0, 1, 3)
        ).astype(bfnp)

    A2OTv = tile_blocks(inputs["a2o_w"])
    O2ATv = tile_blocks(inputs["o2a_w"])
    OUTWTv = tile_blocks(inputs["out_w"])

    def cols(name):
        return np.asarray(inputs[name], np.float32).reshape(HT, P).T

    LNVv = np.zeros((P, 80), np.float32)
    for i, name in enumerate(
        ["ln_o_w", "ln_o_b", "ln_a_w", "ln_a_b", "out_ln1_w", "out_ln1_b",
         "out_ln2_w", "out_ln2_b", "out_b", "lmf_bias"]
    ):
        LNVv[:, 8 * i : 8 * (i + 1)] = cols(name)

    factors = np.asarray(inputs["factors"], np.float32)
    rank_w = np.asarray(inputs["rank_w"], np.float32)
    Ff = factors.copy()
    Ff[AUDIO] = Ff[AUDIO] * rank_w[:, None, None]
    # partition-major layout [R, HT, M, P, 9, P]
    FTv = np.zeros((R, HT, M, P, 9, P), np.float32)
    main = Ff[:, :, 1:, :].reshape(M, R, HT, P, HT, P)   # [m, r, kt, pk, ht, ph]
    FTv[:, :, :, :, :8, :] = main.transpose(1, 4, 0, 3, 2, 5)
    bias = Ff[:, :, 0, :].reshape(M, R, HT, P)           # [m, r, ht, ph]
    FTv[:, :, :, 0, 8, :] = bias.transpose(1, 2, 0, 3)
    ones_row = np.ones((R, HT, M, P), np.float32)
    ones_row[:, :, AUDIO, :] = rank_w[:, None, None]
    FTv[:, :, :, 1, 8, :] = ones_row
    # reorder M to the kernel's chain order (audio last)
    FTv = np.ascontiguousarray(FTv[:, :, [0, 2, 3, 1]])
    # relocate chain mi's K=2 tail rows to partitions 32*mi so the four
    # tails can issue into distinct PE row groups concurrently
    for mi in range(1, M):
        FTv[:, :, mi, 32 * mi : 32 * mi + 2, 8, :] = FTv[:, :, mi, 0:2, 8, :]
        FTv[:, :, mi, 0:2, 8, :] = 0.0
    FTv = FTv.astype(bfnp)

    shared = dict(
        WGO=WGOv.astype(bfnp), WGA=WGAv.astype(bfnp),
        C1=C1v,
        W2=W2v.astype(bfnp), CB=CBv, SC=SCv,
        A2OT=A2OTv, O2AT=O2ATv, OUTWT=OUTWTv, LNV=LNVv, FT=FTv,
    )

    in_maps = []
    for c in range(NCORES):
        pc = perm[:, c]
        tokTv = np.ascontiguousarray(tokens[pc].transpose(1, 2, 0)).astype(bfnp)
        u8v = np.zeros((4, BS), np.uint8)
        u8v[0:3] = pv[pc].T > 0
        u8v[3] = aum[pc] > 0
        f16v = np.zeros((7, BS), np.float32)
        f16v[0:3] = mo[pc].T
        f16v[3] = ma[pc]
        f16v[4:7] = (mo[pc] * winv[pc, None]).T
        uvqv = np.zeros((2 * M, BS), np.float32)
        for mi, m in enumerate(MORD_HOST):  # kernel chain order
            uvqv[2 * mi] = mask_f[pc, m]
            uvqv[2 * mi + 1] = 1.0 - mask_f[pc, m]
        in_maps.append(dict(
            tokT=tokTv, u8rows=u8v, f16rows=f16v.astype(bfnp),
            UVQ=uvqv.astype(bfnp), **shared,
        ))
    return in_maps, perm, runs, gate_runs


def kernel(**inputs):
    global LAST_RESULTS
    in_maps, perm, runs, gate_runs = _host_prep(inputs)
    # full-width gates: the gate matmul time is latency-hiding filler for
    # the gates' serial LN/broadcast chains — narrowing it measured slower.
    # The pure-DVE blend ops do shrink to the pair-valid span (blend_runs).
    blend_runs = gate_runs
    gate_runs = ((0, BS),) * 4
    key = (runs, gate_runs, blend_runs)
    if key not in _cached_nc:
        _cached_nc[key] = _build(runs, gate_runs, blend_runs)
    res = run_bass_kernel_spmd(
        _cached_nc[key], in_maps, core_ids=list(range(NCORES)), trace=TRACE
    )
    LAST_RESULTS = res
    out = np.empty((B, H), np.float32)
    for c in range(NCORES):
        out[perm[:, c]] = np.asarray(res.results[c]["outT"], np.float32).T
    return out



# revision 72
# speedup vs baseline: 1.0179x; 1.0179x over previous
"""TRN2 Bass kernel for nn_BlendEmoBackbone: gated audio mixer + low-rank
multiplicative fusion, data-parallel over batch on 8 NeuronCores.

v3 additions (on top of v2 below):
- Mask-run restriction: host sorts each core's batch columns by the 4-bit
  token-mask pattern (order chosen by an exact subset-DP minimizing the
  weighted chain spans) and deals the globally sorted batch round-robin to
  cores, so every LMF chain's valid columns form one compact run shared by
  all 8 SPMD cores. z == 1 outside the run, so the 8 K-tile matmuls per
  (rank, h-tile, chain) stream only ~64% of the columns; the complement
  columns get their constant from the K=2 tail (kt0's start=True marks the
  whole psum bank pending-zero, so the complement tail writes, not adds).
  The program is compiled per run-tuple at first call (cached).
- LN stats quad-packed: the M=1 ones-matmul column sums issue 4-wide into
  distinct PE column groups (tile_position=(0,32g)) and land in rows
  0/32/64/96 of one psum bank (bf16 only — fp32r rejects col groups).
- Audio-token Σx/Σx² rows computed once and reused by all four gates.
- The four K=2 LMF tails issue into distinct PE row groups
  (tile_position=(32*mi,0); uv rows and factor tail rows replicated at
  partitions 32*mi by the host).
- r0 partial-product chains emitted immediately after the blend that
  unblocks them, filling PE gaps under the later gates' serial chains.

Strategy (v2, bf16):
- Pure data parallel: each core handles B/8 = 512 batch rows; gate MLP
  weights and LMF factor tensors replicated (bf16 halves HBM traffic).
- All activations in transposed [feature, batch] layout; every matmul
  contracts over the partition dim. bf16 operands stream 1 cycle/row on
  the PE (fp32/f32r streams at ~2 cycles/row on real TRN2).
- LayerNorm stats via PE ones-matmul column sums; -mu folded into gate
  matmuls as an extra K=1 row.
- LMF where(mask, z, 1) + x_aug ones-column folded into a K=2 tail tile
  in the same psum chain; rank_w folded into the audio factor slices.
- Factors stored partition-major [R,HT,M,P,9,P] so each (r,ht) loads
  with ONE contiguous-per-partition DMA (2.3KB lines).
- WGO gate weights resident in SBUF (single DMA, reused by 3 gates);
  the audio-source half of the other-gate matmul computed once (S).
- Row->tile broadcasts on gpsimd (partition_broadcast); abs/gelu/
  sigmoid/psum-copies on the scalar engine; products/blends on DVE in
  bf16 where precision allows.
"""

import numpy as np
import ml_dtypes
from contextlib import ExitStack

import concourse.bass as bass
from concourse import bacc
import concourse.tile as tile
from concourse import mybir
from concourse.bass_utils import run_bass_kernel_spmd

B, M, H, R = 4096, 4, 1024, 10
NCORES = 8
BS = B // NCORES          # 512 batch rows per core
MID = 512
P = 128
HT = H // P               # 8 h-tiles
MT = MID // P             # 4 mid-tiles
D3 = 3 * H
OTHERS = (0, 2, 3)
AUDIO = 1
EPS = 1e-5

f32 = mybir.dt.float32
f32r = mybir.dt.float32r
bf16 = mybir.dt.bfloat16
u8 = mybir.dt.uint8
AF = mybir.ActivationFunctionType
OP = mybir.AluOpType
bfnp = ml_dtypes.bfloat16

TRACE = False
LAST_RESULTS = None

_cached_nc = {}


def _build(runs, gate_runs, blend_runs):
    """runs[mi] = (lo, hi): column range covering every batch column where
    LMF chain mi is valid (columns are pre-sorted by mask pattern on the
    host). Outside the run z == 1, so the main k-tile matmuls stream only
    [lo:hi); complement segments get their constant via the K=2 tail.
    gate_runs[j] (j=0..2) covers pair_valid_j; gate_runs[3] covers aum —
    gate math outside its run is discarded by the blend, so all gate
    matmuls/DVE work stream only the run columns."""
    nc = bacc.Bacc("TRN2", target_bir_lowering=False, debug=False)
    # hull of the other-gate runs: a2o / S are consumed inside those runs
    ghull = (min(g[0] for g in gate_runs[:3]), max(g[1] for g in gate_runs[:3]))
    arun = gate_runs[3]

    # ---- DRAM parameters (per core) ----
    tokT = nc.declare_dram_parameter("tokT", [M, H, BS], bf16, isOutput=False)
    # u8 rows: 0-2 pv_j, 3 am(aum)
    u8rows = nc.declare_dram_parameter("u8rows", [4, BS], u8, isOutput=False)
    # bf16 rows: 0-2 mo_j, 3 ma, 4-6 cm_j
    f16rows = nc.declare_dram_parameter("f16rows", [7, BS], bf16, isOutput=False)
    # [mask; 1-mask] pairs per chain mi, placed at partitions 32*mi (+0,1)
    # so the four K=2 LMF tail matmuls can pack into distinct PE row groups.
    UVQ = nc.declare_dram_parameter("UVQ", [2 * M, BS], bf16, isOutput=False)
    WGO = nc.declare_dram_parameter("WGO", [3 * HT, P, MID], bf16, isOutput=False)
    WGA = nc.declare_dram_parameter("WGA", [3 * HT, P, MID], bf16, isOutput=False)
    C1 = nc.declare_dram_parameter("C1", [P, MT, 2], f32, isOutput=False)
    W2 = nc.declare_dram_parameter("W2", [P, MT, 2], bf16, isOutput=False)
    CB = nc.declare_dram_parameter("CB", [P, 8], f32, isOutput=False)
    SC = nc.declare_dram_parameter("SC", [1, 8], f32, isOutput=False)
    # [ht_out, kt, P, P] tiled weight blocks (lhsT layout)
    A2OT = nc.declare_dram_parameter("A2OT", [HT, HT, P, P], bf16, isOutput=False)
    O2AT = nc.declare_dram_parameter("O2AT", [HT, HT, P, P], bf16, isOutput=False)
    OUTWT = nc.declare_dram_parameter("OUTWT", [HT, HT, P, P], bf16, isOutput=False)
    # cols: ln_o_w 0:8, ln_o_b 8:16, ln_a_w 16:24, ln_a_b 24:32,
    #       ln1w 32:40, ln1b 40:48, ln2w 48:56, ln2b 56:64, outb 64:72, lmfb 72:80
    LNV = nc.declare_dram_parameter("LNV", [P, 80], f32, isOutput=False)
    # partition-major factor blocks; [.., p, 0:8, :] = main k-tiles,
    # [.., 0:2, 8, :] = [bias_row; ones_or_rankw_row]
    FT = nc.declare_dram_parameter("FT", [R, HT, M, P, 9, P], bf16, isOutput=False)
    OUT = nc.declare_dram_parameter("outT", [H, BS], bf16, isOutput=True)

    with tile.TileContext(nc) as tc, ExitStack() as ctx:
        kp = ctx.enter_context(tc.tile_pool(name="konst", bufs=1))
        tokp = ctx.enter_context(tc.tile_pool(name="tokp", bufs=1))
        big = ctx.enter_context(tc.tile_pool(name="big", bufs=1))
        wk = ctx.enter_context(tc.tile_pool(name="wk", bufs=2))
        bcp = ctx.enter_context(tc.tile_pool(name="bcp", bufs=1))
        sqp = ctx.enter_context(tc.tile_pool(name="sqp", bufs=9))
        sqf = ctx.enter_context(tc.tile_pool(name="sqf", bufs=3))
        wgp = ctx.enter_context(tc.tile_pool(name="wgp", bufs=2))
        ftp = ctx.enter_context(tc.tile_pool(name="ftp", bufs=2))
        rowp = ctx.enter_context(tc.tile_pool(name="rowp", bufs=1))
        # 3 bufs: the LMF chains' psum WAR on the DVE product reads was the
        # top PE stall in the audio-gate stretch (8/8 PSUM banks now used)
        ppz = ctx.enter_context(tc.tile_pool(name="ppz", bufs=3, space="PSUM"))
        ppg = ctx.enter_context(tc.tile_pool(name="ppg", bufs=4, space="PSUM"))
        pps = ctx.enter_context(tc.tile_pool(name="pps", bufs=1, space="PSUM"))

        # ---- constants / small loads ----
        ones_k = kp.tile([P, 1], bf16)
        nc.vector.memset(ones_k, 1.0)
        ones_kf32 = kp.tile([P, 1], f32)
        nc.vector.memset(ones_kf32, 1.0)
        ones_kf = ones_kf32.bitcast(f32r)

        def bc_row_dma(dst, src_ap):
            nc.sync.dma_start(
                out=dst,
                in_=bass.AP(
                    tensor=src_ap.tensor, offset=src_ap.offset, ap=[[0, P], [1, BS]]
                ),
            )

        # ---- tokens (transposed, bf16); audio first so a2o starts early;
        # tokens 2,3 deferred past the first weight loads (needed later) ----
        tok = tokp.tile([P, M, HT, BS], bf16)
        for m in (AUDIO, 0):
            nc.sync.dma_start(
                out=tok[:, m], in_=tokT.ap()[m].rearrange("(ht p) b -> p ht b", p=P)
            )

        u8t = []
        for i in range(4):
            t = kp.tile([P, BS], u8, tag=f"u8_{i}")
            bc_row_dma(t, u8rows.ap()[i : i + 1, :])
            u8t.append(t)
        pv_t, am_t = u8t[0:3], u8t[3]
        f16t = []
        for i in range(7):
            t = kp.tile([P, BS], bf16, tag=f"f16_{i}")
            bc_row_dma(t, f16rows.ap()[i : i + 1, :])
            f16t.append(t)
        mo_t, ma_t, cm_t = f16t[0:3], f16t[3], f16t[4:7]
        # uvq: chain mi's [mask; 1-mask] rows at partitions 32*mi, 32*mi+1
        uvq = kp.tile([P, BS], bf16)
        for mi in range(M):
            for j in range(2):
                nc.sync.dma_start(
                    out=uvq[32 * mi + j : 32 * mi + j + 1, :],
                    in_=UVQ.ap()[2 * mi + j : 2 * mi + j + 1, :],
                )

        def uvr(mi):
            return uvq[32 * mi : 32 * mi + 2, :]
        cbt = kp.tile([P, 8], f32)
        nc.sync.dma_start(out=cbt, in_=CB.ap())
        sct = kp.tile([1, 8], f32)
        nc.sync.dma_start(out=sct, in_=SC.ap())
        lnv = kp.tile([P, 80], f32)
        nc.sync.dma_start(out=lnv, in_=LNV.ap())
        w2t = kp.tile([P, MT, 2], bf16)
        nc.sync.dma_start(out=w2t, in_=W2.ap())
        c1t = kp.tile([P, MT, 2], f32)
        nc.sync.dma_start(out=c1t, in_=C1.ap())

        def tk(m, kt):
            return tok[:, m, kt, :]

        def tkw(m):  # whole-token [P, HT, BS] view
            return tok[:, m]

        def flat(t3):
            return t3.rearrange("p a b -> p (a b)")

        def b3(t2):  # [P,BS] -> broadcast [P,HT,BS]
            return t2.unsqueeze(1).broadcast_to([P, HT, BS])

        # ---- quad-packed LN stats ----
        # statQ rows 0/32/64/96 are independent psum accumulators; chunk
        # matmuls are issued into distinct PE column groups (tile_position)
        # so up to 4 of them execute concurrently in the array.
        QROW = (0, 32, 64, 96)

        def quad_alloc(name):
            return pps.tile([97, BS], f32, tag="statQ", name=name)

        def quad_mm(statQ, g, rhs, start, stop, lhs=None, cols=(0, BS)):
            r = QROW[g]
            lo, hi = cols
            nc.tensor.matmul(
                statQ[r : r + 1, lo:hi], ones_k if lhs is None else lhs, rhs,
                start=start, stop=stop, tile_position=(0, r),
            )

        def quad_finish(statQ, extraA=None, extraB=None, cols=(0, BS)):
            # DVE may read at most one PSUM operand per instruction
            lo, hi = cols
            rowA = rowp.tile([1, BS], f32, tag="qrowA")
            rowB = rowp.tile([1, BS], f32, tag="qrowB")
            if extraA is not None:
                nc.vector.tensor_add(rowA[:, lo:hi], statQ[0:1, lo:hi], extraA[:, lo:hi])
            else:
                nc.scalar.activation(rowA[:, lo:hi], statQ[0:1, lo:hi], AF.Copy)
            nc.vector.tensor_add(rowA[:, lo:hi], rowA[:, lo:hi], statQ[32:33, lo:hi])
            if extraB is not None:
                nc.vector.tensor_add(rowB[:, lo:hi], statQ[64:65, lo:hi], extraB[:, lo:hi])
            else:
                nc.scalar.activation(rowB[:, lo:hi], statQ[64:65, lo:hi], AF.Copy)
            nc.vector.tensor_add(rowB[:, lo:hi], rowB[:, lo:hi], statQ[96:97, lo:hi])
            return rowA, rowB

        # ---- helpers ----
        def ln_rows(statA, statB, n, tag, hi_mu=False, par=0, cols=(0, BS)):
            lo, hi = cols
            mdt = f32 if hi_mu else bf16
            mtag = "negmuf" if hi_mu else f"negmu{par}"
            negmu = rowp.tile([1, BS], mdt, tag=mtag, name=f"negmu_{tag}")
            nc.scalar.activation(negmu[:, lo:hi], statA[0:1, lo:hi], AF.Copy, bias=0.0, scale=-1.0 / n)
            ex2 = rowp.tile([1, BS], f32, tag="ex2", name=f"ex2_{tag}")
            nc.scalar.activation(ex2[:, lo:hi], statB[0:1, lo:hi], AF.Copy, bias=0.0, scale=1.0 / n)
            msq = rowp.tile([1, BS], f32, tag="msq", name=f"msq_{tag}")
            nc.scalar.activation(msq[:, lo:hi], negmu[:, lo:hi], AF.Square)
            nc.vector.tensor_sub(ex2[:, lo:hi], ex2[:, lo:hi], msq[:, lo:hi])
            nc.vector.tensor_scalar_max(ex2[:, lo:hi], ex2[:, lo:hi], 0.0)
            # 1/sqrt(var+eps) = exp(-0.5*ln(var+eps)); ln+exp share one ACT
            # table so this replaces the 3.3us DVE reciprocal with two fast
            # scalar-engine ops off the DVE queue.
            nc.scalar.activation(msq[:, lo:hi], ex2[:, lo:hi], AF.Ln, bias=sct[0:1, 2:3], scale=1.0)
            rinvb = rowp.tile([1, BS], bf16, tag=f"rinvb{par}", name=f"rinvb_{tag}")
            nc.scalar.activation(rinvb[:, lo:hi], msq[:, lo:hi], AF.Exp, bias=0.0, scale=-0.5)
            return negmu, rinvb

        def bcast(row, tag, dt=bf16, cols=(0, BS)):
            """Broadcast a [1,*] row to [P,*] via gpsimd."""
            lo, hi = cols
            sb = bcp.tile([P, BS], dt, tag=f"bc_{tag}")
            nc.gpsimd.partition_broadcast(sb[:, lo:hi], row[0:1, lo:hi])
            return sb

        def b3c(t2, lo, hi):  # [P,BS] row-tile slice -> [P,HT,cols] bcast view
            return t2[:, lo:hi].unsqueeze(1).broadcast_to([P, HT, hi - lo])

        def linmap(WT, src3, dst3, cols=(0, BS)):
            """dst3[ho] = sum_kt WT[ho,kt].T @ src3[kt]; WT streamed from DRAM."""
            lo, hi = cols
            for ho in range(HT):
                wt = wgp.tile([P, HT, P], bf16, tag="lin")
                nc.sync.dma_start(out=wt, in_=WT.ap()[ho].rearrange("k p c -> p k c"))
                ps = ppz.tile([P, BS], f32, tag="z")
                for kt in range(HT):
                    nc.tensor.matmul(
                        ps[:, lo:hi], wt[:, kt, :], src3[:, kt, lo:hi],
                        start=(kt == 0), stop=(kt == HT - 1),
                    )
                nc.scalar.activation(dst3[:, ho, lo:hi], ps[:, lo:hi], AF.Copy)

        # ---- cached audio column stats (Σx, Σx²), shared by all 4 gates;
        # also primes the PE while the weight DMAs land ----
        csQ = quad_alloc("csQ")
        for k in range(HT):
            asq = sqp.tile([P, BS], bf16, tag="sq_sq", name=f"asq{k}")
            nc.vector.tensor_mul(asq, tk(AUDIO, k), tk(AUDIO, k))
            quad_mm(csQ, k % 2, tk(AUDIO, k), start=(k < 2), stop=(k >= HT - 2))
            quad_mm(csQ, 2 + k % 2, asq, start=(k < 2), stop=(k >= HT - 2))
        cs_a = kp.tile([1, BS], f32)
        ss_a = kp.tile([1, BS], f32)
        nc.scalar.activation(cs_a, csQ[0:1, :], AF.Copy)
        nc.vector.tensor_add(cs_a, cs_a, csQ[32:33, :])
        nc.scalar.activation(ss_a, csQ[64:65, :], AF.Copy)
        nc.vector.tensor_add(ss_a, ss_a, csQ[96:97, :])

        # ---- a2o = audio @ a2o_w.T, in T layout (bf16) ----
        a2or = big.tile([P, HT, BS], bf16, tag="axr")
        linmap(A2OT, tkw(AUDIO), a2or, cols=ghull)

        # S_mt = sum_k Wgo_s[k].T @ audio  (shared source half of gate1)
        hlo, hhi = ghull
        S = big.tile([P, MT, BS], bf16, tag="Sg")
        for mt in range(MT):
            wS = wgp.tile([P, HT, P], bf16, tag="lin", name=f"wS{mt}")
            nc.sync.dma_start(
                out=wS,
                in_=WGO.ap()[HT : 2 * HT, :, mt * P : (mt + 1) * P].rearrange(
                    "k p c -> p k c"
                ),
            )
            ps = ppz.tile([P, BS], f32, tag="z")
            for k in range(HT):
                nc.tensor.matmul(
                    ps[:, hlo:hhi], wS[:, k, :], tk(AUDIO, k)[:, hlo:hhi],
                    start=(k == 0), stop=(k == HT - 1),
                )
            nc.scalar.activation(S[:, mt, hlo:hhi], ps[:, hlo:hhi], AF.Copy)

        # deferred token loads (queued behind the first gate's weights)
        for m in (2, 3):
            nc.sync.dma_start(
                out=tok[:, m], in_=tokT.ap()[m].rearrange("(ht p) b -> p ht b", p=P)
            )

        omt = big.tile([P, HT, BS], bf16, tag="om")  # others_mean accumulator
        mix_src = {"x": a2or}

        def gate_phase1(j, mj):
            """Stats + gate1 matmuls + LN rows for gate j. Emission order
            keeps the PE fed: gate1 halves (no stats dependency) are
            interleaved with the DVE-paced stat chains."""
            is_audio = j == 3
            glo, ghi = gate_runs[j]
            t_m = AUDIO if is_audio else mj
            t3 = tkw(t_m)
            s3 = omt if is_audio else tkw(AUDIO)
            x3 = s3 if is_audio else t3  # the non-cached (non-audio) operand

            abs3 = big.tile([P, HT, BS], bf16, tag="abs", name=f"abs3_{j}")
            statQ = quad_alloc(f"statQ_{j}")
            gps = [
                ppg.tile([P, BS], f32, tag="g", name=f"gps{j}_{mt}")
                for mt in range(MT)
            ]
            if is_audio:
                parts = [(WGA, 0, t3), (WGA, 1, s3), (WGA, 2, abs3)]
            else:
                parts = [(WGO, 0, t3), (WGO, 2, abs3)]

            # DVE production first: d, |d| (ACT), d^2 tiles
            dsq = []
            for k in range(HT):
                dk = wk.tile([P, BS], bf16, tag="dk")
                nc.vector.tensor_sub(dk[:, glo:ghi], t3[:, k, glo:ghi], s3[:, k, glo:ghi])
                nc.scalar.activation(abs3[:, k, glo:ghi], dk[:, glo:ghi], AF.Abs)
                sq = sqp.tile([P, BS], bf16, tag="sq_sq")
                nc.vector.tensor_mul(sq[:, glo:ghi], dk[:, glo:ghi], dk[:, glo:ghi])
                dsq.append(sq)

            def mt_chain(mt):
                for pi, (WG, part, rhs3) in enumerate(parts):
                    w = wgp.tile([P, HT, P], bf16, tag="lin", name=f"wg{j}_{mt}_{part}")
                    nc.sync.dma_start(
                        out=w,
                        in_=WG.ap()[
                            part * HT : (part + 1) * HT, :, mt * P : (mt + 1) * P
                        ].rearrange("k p c -> p k c"),
                    )
                    for k in range(HT):
                        nc.tensor.matmul(
                            gps[mt][:, glo:ghi], w[:, k, :], rhs3[:, k, glo:ghi],
                            start=(pi == 0 and k == 0),
                            stop=(pi == len(parts) - 1 and k == HT - 1),
                        )

            # quad stats: g0=x3 (A), g1=abs3 (A), g2=d^2 (B), g3=x3^2 (B);
            # the audio-token Σx/Σx² parts come from the cached rows.
            def stat_slots(ks):
                for k in ks:
                    sq = sqp.tile([P, BS], bf16, tag="sq_sq", name=f"xsq{j}_{k}")
                    nc.vector.tensor_mul(sq[:, glo:ghi], x3[:, k, glo:ghi], x3[:, k, glo:ghi])
                    quad_mm(statQ, 0, x3[:, k, glo:ghi], start=(k == 0), stop=(k == HT - 1), cols=(glo, ghi))
                    quad_mm(statQ, 1, abs3[:, k, glo:ghi], start=(k == 0), stop=(k == HT - 1), cols=(glo, ghi))
                    quad_mm(statQ, 2, dsq[k][:, glo:ghi], start=(k == 0), stop=(k == HT - 1), cols=(glo, ghi))
                    quad_mm(statQ, 3, sq[:, glo:ghi], start=(k == 0), stop=(k == HT - 1), cols=(glo, ghi))

            mt_chain(0)
            mt_chain(1)
            stat_slots(range(0, HT // 2))
            mt_chain(2)
            stat_slots(range(HT // 2, HT))
            mt_chain(3)
            rowA, rowB = quad_finish(statQ, extraA=cs_a, extraB=ss_a, cols=(glo, ghi))
            negmu, rinvb = ln_rows(rowA, rowB, D3, f"g{j}", par=j % 2, cols=(glo, ghi))
            return abs3, negmu, rinvb, gps

        def gate_phase2a(j, mj, abs3, negmu, rinvb, gps):
            """Gate layer 2 + mix pre-activation for gate j."""
            is_audio = j == 3
            glo, ghi = gate_runs[j]
            t3 = tkw(AUDIO if is_audio else mj)
            rb = bcast(rinvb, "rb", cols=(glo, ghi))
            nmb = bcast(negmu, "nm", cols=(glo, ghi))
            cb_off = 4 if is_audio else 0
            col = 1 if is_audio else 0
            gp = pps.tile([1, BS], f32, tag="statQ", name=f"gp{j}")
            for mt in range(MT):
                hm = wk.tile([P, BS], f32, tag="hm")
                # hm = gps + (-mu)*c1 [+ S]; then * rinv
                nc.vector.scalar_tensor_tensor(
                    hm[:, glo:ghi], nmb[:, glo:ghi], c1t[:, mt, col : col + 1],
                    gps[mt][:, glo:ghi], op0=OP.mult, op1=OP.add,
                )
                if not is_audio:
                    nc.vector.tensor_add(hm[:, glo:ghi], hm[:, glo:ghi], S[:, mt, glo:ghi])
                nc.vector.tensor_mul(hm[:, glo:ghi], hm[:, glo:ghi], rb[:, glo:ghi])
                hg1 = wk.tile([P, BS], bf16, tag="hg", name=f"hg{mt}")
                nc.scalar.activation(
                    hg1[:, glo:ghi], hm[:, glo:ghi], AF.Gelu,
                    bias=cbt[:, cb_off + mt : cb_off + mt + 1], scale=1.0,
                )
                nc.tensor.matmul(
                    gp[0:1, glo:ghi], w2t[:, mt, col : col + 1], hg1[:, glo:ghi],
                    start=(mt == 0), stop=(mt == MT - 1),
                )
            g_row = rowp.tile([1, BS], bf16, tag="g_row")
            nc.scalar.activation(
                g_row[:, glo:ghi], gp[0:1, glo:ghi], AF.Sigmoid,
                bias=sct[0:1, col : col + 1], scale=1.0,
            )
            gb = bcast(g_row, "gb", cols=(glo, ghi))
            # pre = t + g * (a2o | o2a)
            src = mix_src["x"]
            pre = big.tile([P, HT, BS], bf16, tag=f"pre{j % 2}", name=f"pre{j}")
            nc.vector.tensor_mul(pre[:, :, glo:ghi], src[:, :, glo:ghi], b3c(gb, glo, ghi))
            nc.vector.tensor_add(pre[:, :, glo:ghi], pre[:, :, glo:ghi], t3[:, :, glo:ghi])
            return pre

        def gate_phase2b(j, mj, pre):
            """Mix LN + blend for gate j."""
            is_audio = j == 3
            glo, ghi = gate_runs[j]
            alo, ahi = arun
            t_m = AUDIO if is_audio else mj
            t3 = tkw(t_m)
            stat2Q = quad_alloc(f"stat2Q_{j}")
            for k in range(HT):
                sq = sqp.tile([P, BS], bf16, tag="sq_sq", name=f"psq{j}_{k}")
                nc.vector.tensor_mul(sq[:, glo:ghi], pre[:, k, glo:ghi], pre[:, k, glo:ghi])
                quad_mm(stat2Q, k % 2, pre[:, k, glo:ghi], start=(k < 2), stop=(k >= HT - 2), cols=(glo, ghi))
                quad_mm(stat2Q, 2 + k % 2, sq[:, glo:ghi], start=(k < 2), stop=(k >= HT - 2), cols=(glo, ghi))
            rowA2, rowB2 = quad_finish(stat2Q, cols=(glo, ghi))
            negmu2, rinvb2 = ln_rows(rowA2, rowB2, H, f"u{j}", par=2 + (j % 2), cols=(glo, ghi))
            mb = bcast(negmu2, "mb", cols=(glo, ghi))
            rb2 = bcast(rinvb2, "rb2", cols=(glo, ghi))
            wcol = 16 if is_audio else 0
            bcol = 24 if is_audio else 8
            sm = am_t if is_audio else pv_t[j]
            bmf = ma_t if is_audio else mo_t[j]
            # whole-token LN apply + blend: tok = bmf * (sm ? ln(pre) : t).
            # q3 is only ever SELECTED where sm (pair_valid) is set, and the
            # sorted columns confine sm to blend_runs[j] — so the LN apply
            # and the predicated copy run only that span (pure DVE relief;
            # the gate matmuls stay full width as latency filler).
            qlo, qhi = blend_runs[j]
            q3 = big.tile([P, HT, BS], bf16, tag="q3", name=f"q3_{j}")
            nc.vector.tensor_add(q3[:, :, qlo:qhi], pre[:, :, qlo:qhi], b3c(mb, qlo, qhi))
            nc.vector.tensor_mul(q3[:, :, qlo:qhi], q3[:, :, qlo:qhi], b3c(rb2, qlo, qhi))
            for kt in range(HT):
                nc.vector.tensor_scalar(
                    q3[:, kt, qlo:qhi], q3[:, kt, qlo:qhi],
                    lnv[:, wcol + kt : wcol + kt + 1], lnv[:, bcol + kt : bcol + kt + 1],
                    op0=OP.mult, op1=OP.add,
                )
            nc.vector.copy_predicated(t3[:, :, qlo:qhi], b3c(sm, qlo, qhi), q3[:, :, qlo:qhi])
            nc.vector.tensor_mul(t3, t3, b3(bmf))
            if not is_audio:
                # others_mean is only consumed inside the aum run
                if j == 0:
                    nc.vector.tensor_mul(omt[:, :, alo:ahi], t3[:, :, alo:ahi], b3c(cm_t[j], alo, ahi))
                else:
                    tmp3 = big.tile([P, HT, BS], bf16, tag="q3", name=f"omtmp_{j}")
                    nc.vector.tensor_mul(tmp3[:, :, alo:ahi], t3[:, :, alo:ahi], b3c(cm_t[j], alo, ahi))
                    nc.vector.tensor_add(omt[:, :, alo:ahi], omt[:, :, alo:ahi], tmp3[:, :, alo:ahi])

        # LMF chain-index order: factors stored with M reordered as MORD so
        # the audio (blended last) chain comes last; r=0 partial products
        # for the non-audio tokens are emitted between mixer phases to keep
        # the PE busy during the gates' serial post-chains.
        MORD = (0, 2, 3, 1)
        acc = big.tile([P, HT, BS], f32r, tag="acc")
        soth = big.tile([P, HT, BS], bf16, tag="soth")

        def chain_tails(zp, ft_tail, mi, concurrent_ok=True):
            """K=2 tail over the run (accumulate) plus complement segments
            (first write there — kt0's start marked the bank pending-zero,
            so these overwrite with mask*bias + (1-mask)*1 == 1)."""
            lo, hi = runs[mi]
            segs = [(lo, hi)]
            if lo > 0:
                segs.append((0, lo))
            if hi < BS:
                segs.append((hi, BS))
            for si, (s, e) in enumerate(segs):
                nc.tensor.matmul(
                    zp[:, s:e], ft_tail, uvr(mi)[:, s:e],
                    start=False, stop=(si == len(segs) - 1),
                    tile_position=(32 * mi, 0),
                )

        def lmf_part_chain(r, mi, dst):
            """Partial-product chains for rank r emitted out of band: dst
            accumulates z products for chain mi; mi==3 folds into acc."""
            lo, hi = runs[mi]
            for ht in range(HT):
                ft = ftp.tile([P, 9, P], bf16, tag="ft0", bufs=6)
                nc.sync.dma_start(out=ft, in_=FT.ap()[r, ht, mi])
                zp = ppz.tile([P, BS], f32, tag="z")
                for kt in range(HT):
                    nc.tensor.matmul(
                        zp[:, lo:hi], ft[:, kt, :], tk(MORD[mi], kt)[:, lo:hi],
                        start=(kt == 0), stop=False,
                    )
                chain_tails(zp, ft[32 * mi : 32 * mi + 2, 8, :], mi)
                if mi == 0:
                    nc.scalar.activation(dst[:, ht, :], zp, AF.Copy)
                elif mi < 3:
                    nc.vector.tensor_mul(dst[:, ht, :], dst[:, ht, :], zp)
                elif r == 0:
                    nc.vector.tensor_mul(acc[:, ht, :], dst[:, ht, :], zp)
                else:
                    s0 = wk.tile([P, BS], f32, tag="s0")
                    nc.vector.tensor_mul(s0, dst[:, ht, :], zp)
                    nc.vector.tensor_add(acc[:, ht, :], acc[:, ht, :], s0)

        # software-pipelined emission: gate j+1's stats+gate1 overlap gate
        # j's post-matmul chain on the PE; LMF r=0 chains fill blend windows.
        p1, p2 = {}, {}
        p1[0] = gate_phase1(0, OTHERS[0])
        p1[1] = gate_phase1(1, OTHERS[1])
        p2[0] = gate_phase2a(0, OTHERS[0], *p1[0])
        p1[2] = gate_phase1(2, OTHERS[2])
        p2[1] = gate_phase2a(1, OTHERS[1], *p1[1])
        gate_phase2b(0, OTHERS[0], p2[0])
        # r0 chains emitted as soon as their token is blended: they are the
        # PE filler for the later gates' serial LN/broadcast chains.
        lmf_part_chain(0, 0, soth)
        p2[2] = gate_phase2a(2, OTHERS[2], *p1[2])
        gate_phase2b(1, OTHERS[1], p2[1])
        lmf_part_chain(0, 1, soth)
        gate_phase2b(2, OTHERS[2], p2[2])

        # ---- o2a = others_mean @ o2a_w.T (consumed inside the aum run) ----
        o2ar = big.tile([P, HT, BS], bf16, tag="axr", name="o2ar")
        linmap(O2AT, omt, o2ar, cols=arun)
        mix_src["x"] = o2ar

        p1[3] = gate_phase1(3, AUDIO)
        lmf_part_chain(0, 2, soth)
        p2[3] = gate_phase2a(3, AUDIO, *p1[3])
        # r=1 partial products fill the audio post-chain/blend window
        soth1 = big.tile([P, HT, BS], bf16, tag="axr", name="soth1")
        lmf_part_chain(1, 0, soth1)
        gate_phase2b(3, AUDIO, p2[3])
        lmf_part_chain(1, 1, soth1)
        lmf_part_chain(1, 2, soth1)
        lmf_part_chain(0, 3, soth)
        lmf_part_chain(1, 3, soth1)

        # ---- LMF ranks 2..R-1; LN1 stats interleaved into the last rank ----
        # (fp32r matmuls reject col-group tiling, so A/B go to two banks)
        stat3A = pps.tile([1, BS], f32, tag="statQ", name="stat3A")
        stat3B = ppz.tile([1, BS], f32, tag="z", name="stat3B")
        acb = big.tile([P, HT, BS], bf16, tag="soth", name="acb")

        def stat3_for(ht):
            nc.vector.tensor_scalar_add(
                acc[:, ht, :], acc[:, ht, :], lnv[:, 72 + ht : 72 + ht + 1]
            )
            sq = sqf.tile([P, BS], f32r, tag="sq_f")
            nc.vector.tensor_mul(sq, acc[:, ht, :], acc[:, ht, :])
            nc.scalar.activation(acb[:, ht, :], acc[:, ht, :], AF.Copy)
            nc.tensor.matmul(stat3A, ones_kf, acc[:, ht, :],
                             start=(ht == 0), stop=(ht == HT - 1))
            nc.tensor.matmul(stat3B, ones_kf, sq,
                             start=(ht == 0), stop=(ht == HT - 1))

        for r in range(2, R):
            last = r == R - 1
            for ht in range(HT):
                # per-chain factor DMAs: 4x queue parallelism and chain m's
                # matmuls only wait on their own quarter of the load
                fts = []
                for m in range(M):
                    ftm = ftp.tile([P, 9, P], bf16, tag="ft", bufs=8,
                                   name=f"ft{r}_{ht}_{m}")
                    nc.sync.dma_start(out=ftm, in_=FT.ap()[r, ht, m])
                    fts.append(ftm)
                zps = []
                for m in range(M):
                    zp = ppg.tile([P, BS], f32, tag="g", name=f"zp{r}_{ht}_{m}")
                    lo, hi = runs[m]
                    for kt in range(HT):
                        nc.tensor.matmul(
                            zp[:, lo:hi], fts[m][:, kt, :],
                            tk(MORD[m], kt)[:, lo:hi],
                            start=(kt == 0), stop=False,
                        )
                    zps.append(zp)
                # the K=2 tails pack into distinct PE row groups
                for m in range(M):
                    chain_tails(zps[m], fts[m][32 * m : 32 * m + 2, 8, :], m)
                s0 = wk.tile([P, BS], f32, tag="s0")
                nc.scalar.activation(s0, zps[0], AF.Copy)
                nc.vector.tensor_mul(s0, s0, zps[1])
                nc.vector.tensor_mul(s0, s0, zps[2])
                nc.vector.tensor_mul(s0, s0, zps[3])
                nc.vector.tensor_add(acc[:, ht, :], acc[:, ht, :], s0)
                if last and ht >= 1:
                    stat3_for(ht - 1)
        stat3_for(HT - 1)
        negmu3, rinvb3 = ln_rows(stat3A, stat3B, H, "l1")
        mb3 = bcast(negmu3, "mb")
        rb3 = bcast(rinvb3, "rb2")
        h1 = big.tile([P, HT, BS], bf16, tag="pre0", name="h1")
        nc.vector.tensor_add(h1, acb, b3(mb3))
        nc.vector.tensor_mul(h1, h1, b3(rb3))
        for kt in range(HT):
            nc.vector.tensor_scalar(
                h1[:, kt, :], h1[:, kt, :],
                lnv[:, 32 + kt : 32 + kt + 1], lnv[:, 40 + kt : 40 + kt + 1],
                op0=OP.mult, op1=OP.add,
            )

        # h2 = gelu(h1 @ out_w.T + out_b); LN2 stats interleaved per ho
        h2 = big.tile([P, HT, BS], bf16, tag="abs", name="h2")
        stat4Q = quad_alloc("stat4Q")
        for ho in range(HT):
            wt = wgp.tile([P, HT, P], bf16, tag="lin", name=f"ow{ho}")
            nc.sync.dma_start(out=wt, in_=OUTWT.ap()[ho].rearrange("k p c -> p k c"))
            ps = ppz.tile([P, BS], f32, tag="z")
            for kt in range(HT):
                nc.tensor.matmul(
                    ps, wt[:, kt, :], h1[:, kt, :],
                    start=(kt == 0), stop=(kt == HT - 1),
                )
            nc.scalar.activation(
                h2[:, ho, :], ps, AF.Gelu, bias=lnv[:, 64 + ho : 64 + ho + 1], scale=1.0
            )
            sq4 = sqp.tile([P, BS], bf16, tag="sq_sq", name=f"hsq{ho}")
            nc.vector.tensor_mul(sq4, h2[:, ho, :], h2[:, ho, :])
            quad_mm(stat4Q, ho % 2, h2[:, ho, :], start=(ho < 2), stop=(ho >= HT - 2))
            quad_mm(stat4Q, 2 + ho % 2, sq4, start=(ho < 2), stop=(ho >= HT - 2))
        rowA4, rowB4 = quad_finish(stat4Q)
        negmu4, rinvb4 = ln_rows(rowA4, rowB4, H, "l2")
        mb4 = bcast(negmu4, "mb")
        rb4 = bcast(rinvb4, "rb2")
        fin3 = big.tile([P, HT, BS], bf16, tag="q3", name="fin3")
        nc.vector.tensor_add(fin3, h2, b3(mb4))
        nc.vector.tensor_mul(fin3, fin3, b3(rb4))
        for kt in range(HT):
            nc.vector.tensor_scalar(
                fin3[:, kt, :], fin3[:, kt, :],
                lnv[:, 48 + kt : 48 + kt + 1], lnv[:, 56 + kt : 56 + kt + 1],
                op0=OP.mult, op1=OP.add,
            )
            nc.sync.dma_start(out=OUT.ap()[kt * P : (kt + 1) * P, :], in_=fin3[:, kt, :])

    nc.compile()
    return nc


MORD_HOST = (0, 2, 3, 1)  # kernel chain order (audio last)


def _optimal_order(wp, chain_defs, cw):
    """Order the 16 mask patterns to minimize the weighted sum of chain
    column spans (exact subset DP, maximizes prefix/suffix zero weight)."""
    NP = 16
    FULL = (1 << NP) - 1
    f = [-1.0] * (1 << NP)
    f[0] = 0.0
    parent = [-1] * (1 << NP)
    for S in range(1 << NP):
        if f[S] < 0:
            continue
        base = f[S]
        for p in range(NP):
            bit = 1 << p
            if S & bit:
                continue
            g = 0.0
            for c, Bc in enumerate(chain_defs):
                if not (Bc >> p) & 1:
                    if (S & Bc) == 0 or (S & Bc) == Bc:
                        g += cw[c] * wp[p]
            nS = S | bit
            v = base + g
            if v > f[nS]:
                f[nS] = v
                parent[nS] = p
    order = []
    S = FULL
    while S:
        p = parent[S]
        order.append(p)
        S &= ~(1 << p)
    order.reverse()
    return order


def _plan_columns(token_mask):
    """Sort batch columns by mask pattern (dealt round-robin to cores) so
    each LMF chain's valid columns sit in one compact run per core.
    Returns (perm[BS, NCORES] global indices, runs, gate_runs)."""
    pat = np.zeros(B, dtype=np.int64)
    for mi, m in enumerate(MORD_HOST):
        pat |= np.asarray(token_mask)[:, m].astype(np.int64) << mi
    wp = (np.bincount(pat, minlength=16).astype(np.float64) / B).tolist()
    chain_defs = [sum(1 << p for p in range(16) if (p >> mi) & 1) for mi in range(4)]
    cw = [720.0] * 4
    for j in range(3):  # gate pv_j = chain j valid & audio(chain 3) valid
        chain_defs.append(
            sum(1 << p for p in range(16) if ((p >> j) & 1) and ((p >> 3) & 1))
        )
        cw.append(8.0)
    chain_defs.append(sum(1 << p for p in range(16) if ((p >> 3) & 1) and (p & 7)))
    cw.append(16.0)
    pord = _optimal_order(wp, chain_defs, cw)
    prio = np.zeros(16, dtype=np.int64)
    for pos, p in enumerate(pord):
        prio[p] = pos
    G = np.argsort(prio[pat], kind="stable")
    perm = G.reshape(BS, NCORES)

    def runspan(valid):
        lo = min(int(np.argmax(valid[:, c])) for c in range(NCORES))
        hi = max(BS - int(np.argmax(valid[::-1, c])) for c in range(NCORES))
        return (lo, hi)

    pv = pat[perm]
    runs = tuple(runspan((pv >> mi) & 1) for mi in range(4))
    gate_runs = tuple(
        runspan(((pv >> j) & 1) & ((pv >> 3) & 1)) for j in range(3)
    ) + (runspan(((pv >> 3) & 1) & (pv & 7 > 0)),)
    return perm, runs, gate_runs


def _host_prep(inputs):
    tokens = np.asarray(inputs["tokens"], np.float32)
    token_mask = np.asarray(inputs["token_mask"])
    mask_f = token_mask.astype(np.float32)
    perm, runs, gate_runs = _plan_columns(token_mask)

    mo = mask_f[:, list(OTHERS)]                      # [B,3]
    ma = mask_f[:, AUDIO]                             # [B]
    pv = mo * ma[:, None]                             # [B,3]
    winv = (1.0 / np.clip(mo.sum(1), 1.0, None)).astype(np.float32)
    aum = ma * (mo.max(1) > 0)                        # [B]

    go_w1 = np.asarray(inputs["go_w1"], np.float32)
    ga_w1 = np.asarray(inputs["ga_w1"], np.float32)

    def gate_prep(w1, b1, lnw, lnb):
        W1w = w1 * lnw[None, :]                       # [MID, 3H]
        c1 = np.ascontiguousarray(W1w.sum(1).reshape(1, MID))
        cb = w1 @ lnb + b1                            # [MID]
        Wblocks = np.ascontiguousarray(W1w.T).reshape(3 * HT, P, MID)
        return Wblocks, c1, cb

    WGOv, c1go, cbgo = gate_prep(
        go_w1, np.asarray(inputs["go_b1"], np.float32),
        np.asarray(inputs["ln_go_w"], np.float32), np.asarray(inputs["ln_go_b"], np.float32),
    )
    WGAv, c1ga, cbga = gate_prep(
        ga_w1, np.asarray(inputs["ga_b1"], np.float32),
        np.asarray(inputs["ln_ga_w"], np.float32), np.asarray(inputs["ln_ga_b"], np.float32),
    )
    CBv = np.ascontiguousarray(
        np.concatenate([cbgo.reshape(MT, P).T, cbga.reshape(MT, P).T], axis=1)
    ).astype(np.float32)                              # [P, 8]
    W2v = np.stack(
        [np.asarray(inputs["go_w2"], np.float32).reshape(MID),
         np.asarray(inputs["ga_w2"], np.float32).reshape(MID)], axis=1
    )                                                 # [MID, 2]
    W2v = np.ascontiguousarray(W2v.reshape(MT, P, 2).transpose(1, 0, 2))
    C1v = np.stack([c1go.reshape(MID), c1ga.reshape(MID)], axis=1)
    C1v = np.ascontiguousarray(C1v.reshape(MT, P, 2).transpose(1, 0, 2)).astype(np.float32)
    SCv = np.zeros((1, 8), np.float32)
    SCv[0, 0] = np.asarray(inputs["go_b2"], np.float32).reshape(-1)[0]
    SCv[0, 1] = np.asarray(inputs["ga_b2"], np.float32).reshape(-1)[0]
    SCv[0, 2] = EPS

    def tile_blocks(w):
        wt = np.ascontiguousarray(np.asarray(w, np.float32).T)    # [H_in, H_out]
        return np.ascontiguousarray(
            wt.reshape(HT, P, HT, P).transpose(2, 0, 1, 3)
        ).astype(bfnp)

    A2OTv = tile_blocks(inputs["a2o_w"])
    O2ATv = tile_blocks(inputs["o2a_w"])
    OUTWTv = tile_blocks(inputs["out_w"])

    def cols(name):
        return np.asarray(inputs[name], np.float32).reshape(HT, P).T

    LNVv = np.zeros((P, 80), np.float32)
    for i, name in enumerate(
        ["ln_o_w", "ln_o_b", "ln_a_w", "ln_a_b", "out_ln1_w", "out_ln1_b",
         "out_ln2_w", "out_ln2_b", "out_b", "lmf_bias"]
    ):
        LNVv[:, 8 * i : 8 * (i + 1)] = cols(name)

    factors = np.asarray(inputs["factors"], np.float32)
    rank_w = np.asarray(inputs["rank_w"], np.float32)
    Ff = factors.copy()
    Ff[AUDIO] = Ff[AUDIO] * rank_w[:, None, None]
    # partition-major layout [R, HT, M, P, 9, P]
    FTv = np.zeros((R, HT, M, P, 9, P), np.float32)
    main = Ff[:, :, 1:, :].reshape(M, R, HT, P, HT, P)   # [m, r, kt, pk, ht, ph]
    FTv[:, :, :, :, :8, :] = main.transpose(1, 4, 0, 3, 2, 5)
    bias = Ff[:, :, 0, :].reshape(M, R, HT, P)           # [m, r, ht, ph]
    FTv[:, :, :, 0, 8, :] = bias.transpose(1, 2, 0, 3)
    ones_row = np.ones((R, HT, M, P), np.float32)
    ones_row[:, :, AUDIO, :] = rank_w[:, None, None]
    FTv[:, :, :, 1, 8, :] = ones_row
    # reorder M to the kernel's chain order (audio last)
    FTv = np.ascontiguousarray(FTv[:, :, [0, 2, 3, 1]])
    # relocate chain mi's K=2 tail rows to partitions 32*mi so the four
    # tails can issue into distinct PE row groups concurrently
    for mi in range(1, M):
        FTv[:, :, mi, 32 * mi : 32 * mi + 2, 8, :] = FTv[:, :, mi, 0:2, 8, :]
        FTv[:, :, mi, 0:2, 8, :] = 0.0
    FTv = FTv.astype(bfnp)

    shared = dict(
        WGO=WGOv.astype(bfnp), WGA=WGAv.astype(bfnp),
        C1=C1v,
        W2=W2v.astype(bfnp), CB=CBv, SC=SCv,
        A2OT=A2OTv, O2AT=O2ATv, OUTWT=OUTWTv, LNV=LNVv, FT=FTv,
    )

    in_maps = []
    for c in range(NCORES):
        pc = perm[:, c]
        tokTv = np.ascontiguousarray(tokens[pc].transpose(1, 2, 0)).astype(bfnp)
        u8v = np.zeros((4, BS), np.uint8)
        u8v[0:3] = pv[pc].T > 0
        u8v[3] = aum[pc] > 0
        f16v = np.zeros((7, BS), np.float32)
        f16v[0:3] = mo[pc].T
        f16v[3] = ma[pc]
        f16v[4:7] = (mo[pc] * winv[pc, None]).T
        uvqv = np.zeros((2 * M, BS), np.float32)
        for mi, m in enumerate(MORD_HOST):  # kernel chain order
            uvqv[2 * mi] = mask_f[pc, m]
            uvqv[2 * mi + 1] = 1.0 - mask_f[pc, m]
        in_maps.append(dict(
            tokT=tokTv, u8rows=u8v, f16rows=f16v.astype(bfnp),
            UVQ=uvqv.astype(bfnp), **shared,
        ))
    return in_maps, perm, runs, gate_runs


def kernel(**inputs):
    global LAST_RESULTS
    in_maps, perm, runs, gate_runs = _host_prep(inputs)
    # full-width gates: the gate matmul time is latency-hiding filler for
    # the gates' serial LN/broadcast chains — narrowing it measured slower.
    # The pure-DVE blend ops do shrink to the pair-valid span (blend_runs).
    blend_runs = gate_runs
    gate_runs = ((0, BS),) * 4
    key = (runs, gate_runs, blend_runs)
    if key not in _cached_nc:
        _cached_nc[key] = _build(runs, gate_runs, blend_runs)
    res = run_bass_kernel_spmd(
        _cached_nc[key], in_maps, core_ids=list(range(NCORES)), trace=TRACE
    )
    LAST_RESULTS = res
    out = np.empty((B, H), np.float32)
    for c in range(NCORES):
        out[perm[:, c]] = np.asarray(res.results[c]["outT"], np.float32).T
    return out

